# revision 34
# baseline (speedup 1.0000x reference)
"""Trainium2 Bass kernel for nn_LogicConv3d (DiffLogic conv tree).

Per-call wall clock is dominated by the axon tunnel (~70ms fixed RTT +
~25-55MB/s for incompressible bytes), so the design minimizes wire bytes
and round trips; device compute (~1.5ms) is irrelevant by comparison.

Fast path (structured conv indices AND x in [0,1]):
  - Shard num_kernels K=64 across 8 cores (8 kernels/core).
  - Uploads per call: x as u16 fixed point (98KB/core; the 1/65535
    dequant scale is folded into the L0 coefficient sets), per-core u8
    one-hot L0 gather matrices (zstd on the wire crushes one-hot u8),
    per-core f32 coefficient sets (22KB), u8 zero output buffers
    (compress to ~nothing). The 24 level-shuffle matrices and the 25
    channel->window-row scatter matrices are baked into the NEFF as
    constants (inline_tensor).
  - Device, per batch: contiguous DMA of x[b] ([3,1024] u16), DVE
    dequantize to f32, 25 strided window copies ([3,784] views at shift
    (dh,dw)), one-hot scatter matmuls accumulate them into the [75,784]
    im2col window tile in PSUM. (Overlapping-stride DMA descriptors
    corrupt on this HW for most src offsets -- engine ops only.)
    Then 7 tree levels:
      A,B = PE one-hot selection matmuls (even/odd child shuffle)
      u = c3*A + c2; v = c1*A + c0 (ScalarE), state = u*B + v (VectorE)
    Deep levels (3-6) pack batches into partitions to keep lanes full.
  - Output: [128=(b16,k8), 784] u8 per core; 254*y + 0.5 is folded into
    the L6 coefficient set (y in [0,1] exactly: all 16 gates map [0,1]^2
    into [0,1] and softmax mixes are convex); host decodes (q-0.5)/254.
    Max abs error ~2e-3 vs the 2e-2 gate.
  - Dispatch through a cached jax.jit(shard_map(bass_exec)) callable so
    repeat calls skip retracing; outputs fetched immediately after the
    async dispatch so the d2h request overlaps the dispatch round trip.
  - Host preprocessing is memoized: the full structure check runs once
    per distinct index-buffer pair, coefficient folding once per weight
    bytes.

Fallback (arbitrary indices or x outside [0,1]): host gather + the
original full-precision Ain/Bin program via run_bass_kernel_spmd.
"""

import numpy as np

B, C, H, W = 16, 3, 32, 32
K = 64
RF = 5
DEPTH = 6
S = 2 ** DEPTH          # 64
P = 784                 # 28*28 conv positions
NCORES = 8
KLOC = K // NCORES      # 8 kernels per core
COLS = [(0, 512), (512, 784)]   # fp32 matmul moving-dim <= 512
XLEN = 27 * 32 + 27 + 1         # 892: max in-window flat offset + 1

_GATE_COEFFS = np.array([
    [0, 0, 0, 0], [0, 0, 0, 1], [0, 1, 0, -1], [0, 1, 0, 0],
    [0, 0, 1, -1], [0, 0, 1, 0], [0, 1, 1, -2], [0, 1, 1, -1],
    [1, -1, -1, 1], [1, -1, -1, 2], [1, 0, -1, 0], [1, 0, -1, 1],
    [1, -1, 0, 0], [1, -1, 0, 1], [1, 0, 0, -1], [1, 0, 0, 0],
], dtype=np.float32)


def _softmax(x, axis=-1):
    x = x - x.max(axis=axis, keepdims=True)
    e = np.exp(x)
    return e / e.sum(axis=axis, keepdims=True)


def _coeffs(w):
    """w: [S_l, K, 16] -> [S_l, K, 4] polynomial coefficients."""
    return _softmax(w.astype(np.float64)).astype(np.float32) @ _GATE_COEFFS


def build_sel_mats():
    """24 one-hot matrices [6 levels][side 2][rel 2][128 rows(src), 128 cols(dst)].

    Level l in 1..6 consumes state_{l-1}; dst tile column j maps to a source
    row in one of two source tile instances (rel 0/1). Patterns are shared
    across batches / dst-tile instances by construction.
    """
    mats = np.zeros((6, 2, 2, 128, 128), dtype=np.float32)

    def put(l, rel, row, j):
        mats[l - 1, 0, rel, row, j] = 1.0      # A side (even child)
        mats[l - 1, 1, rel, row + 1, j] = 1.0  # B side (odd child = row+1)

    for j in range(128):
        # L1: dst id=128d+j = kloc*32+t, kloc=4d+j//32 ; src id = kloc*64+2t
        k, t = j // 32, j % 32
        put(1, k // 2, (k % 2) * 64 + 2 * t, j)
        # L2: kloc=j//16, t=j%16 ; src id = kloc*32+2t (256 nodes, 2 tiles)
        k, t = j // 16, j % 16
        put(2, k // 4, (k % 4) * 32 + 2 * t, j)
        # L3: dst (bhat=j//64, id=j%64=k*8+t); src = per-batch state2[bhat]
        bh, idd = j // 64, j % 64
        k, t = idd // 8, idd % 8
        put(3, bh, k * 16 + 2 * t, j)
        # L4: dst (bhat=j//32, id=k*4+t); src state3 packed nb=2
        bh, idd = j // 32, j % 32
        k, t = idd // 4, idd % 4
        put(4, bh // 2, (bh % 2) * 64 + k * 8 + 2 * t, j)
        # L5: dst (bhat=j//16, id=k*2+t); src state4 packed nb=4
        bh, idd = j // 16, j % 16
        k, t = idd // 2, idd % 2
        put(5, bh // 4, (bh % 4) * 32 + k * 4 + 2 * t, j)
        # L6: dst (bhat=j//8, k=j%8); src state5 packed nb=8
        bh, k = j // 8, j % 8
        put(6, bh // 8, (bh % 8) * 16 + k * 2, j)
    return mats


def build_coef_sets(coefs, core):
    """11 coefficient sets [128, 4] for one core (kernels core*8..core*8+7).

    Sets: 0-3 L0 tiles g0..g3; 4-5 L1 d0,d1; 6 L2; 7-10 L3..L6.
    coefs: list of 7 arrays [S_l, K, 4].
    """
    k0 = core * KLOC
    out = np.zeros((11, 128, 4), dtype=np.float32)
    r = np.arange(128)
    for g in range(4):
        out[g] = coefs[0][r % 64, k0 + 2 * g + r // 64]
    for d in range(2):
        out[4 + d] = coefs[1][r % 32, k0 + 4 * d + r // 32]
    out[6] = coefs[2][r % 16, k0 + r // 16]
    out[7] = coefs[3][(r % 64) % 8, k0 + (r % 64) // 8]
    out[8] = coefs[4][(r % 32) % 4, k0 + (r % 32) // 4]
    out[9] = coefs[5][(r % 16) % 2, k0 + (r % 16) // 2]
    out[10] = coefs[6][0, k0 + r % 8]
    return out


def detect_structure(left_idx, right_idx):
    """If idx[k,p,s] = window_base[k,s] + conv_offset[p] (as produced by the
    reference's setup_inputs), return (widxL, widxR): [K, S] window ids in
    [0, 75) = (c*5+dh)*5+dw. Else None."""
    poff = ((np.arange(28, dtype=np.int32)[:, None] * W
             + np.arange(28, dtype=np.int32)[None, :]).ravel())
    ph, pw = poff // W, poff % W                          # [P]
    pvec = np.stack([ph, pw, np.zeros_like(ph)], axis=-1)  # [P, 3]
    out = []
    for idx in (left_idx, right_idx):
        idx = idx.astype(np.int32, copy=False)
        base = idx[:, 0, :, :]                            # [K, S, 3] (p=0)
        hb, wb, cb = base[..., 0], base[..., 1], base[..., 2]
        if (base.min() < 0 or hb.max() >= RF or wb.max() >= RF
                or cb.max() >= C):
            return None
        if not np.array_equal(
                idx, base[:, None, :, :] + pvec[None, :, None, :]):
            return None
        out.append((cb * RF * RF + hb * RF + wb).astype(np.int64))  # [K, S]
    return out


_IDX_MEMO = {}


def detect_structure_memo(left_idx, right_idx):
    """Memoized structure check. Keyed on array identity plus a strided
    content sample; full check on first sight of a buffer pair."""
    key = (id(left_idx), id(right_idx), left_idx.shape, right_idx.shape)
    samp = (left_idx[::13, ::17, ::7].tobytes(),
            right_idx[::13, ::17, ::7].tobytes())
    hit = _IDX_MEMO.get(key)
    if hit is not None and hit[0] == samp:
        return hit[1]
    widx = detect_structure(left_idx, right_idx)
    _IDX_MEMO[key] = (samp, widx)
    return widx


def build_sel0_all(widx):
    """[NCORES, 8, 75, 128] u8 one-hot L0 gather matrices, all cores.

    mat[c, g*2+side][row=window id, col=(k2=j//64, s=j%64)] selects the
    leaf window for kernel c*8+2g+(j//64), leaf s."""
    widxL, widxR = widx
    out = np.zeros((NCORES, 8, 75, 128), dtype=np.uint8)
    j = np.arange(128)
    for c in range(NCORES):
        for g in range(4):
            kg = c * KLOC + 2 * g + j // 64
            out[c, 2 * g, widxL[kg, j % 64], j] = 1
            out[c, 2 * g + 1, widxR[kg, j % 64], j] = 1
    return out


# ---------------------------------------------------------------- device ----

_CACHE = {}


def _build_bass_fast():
    """Structured-path program: consumes x directly, builds windows on
    device, level-shuffle matrices baked in as NEFF constants."""
    import concourse.mybir as mybir
    from concourse import bacc
    from concourse.tile import TileContext
    from bass_rust import AP

    f32 = mybir.dt.float32
    u8 = mybir.dt.uint8
    u16 = mybir.dt.uint16
    Ident = mybir.ActivationFunctionType.Identity

    nc = bacc.Bacc("TRN2", target_bir_lowering=False, debug=False,
                   num_devices=NCORES)
    # x quantized to u16 fixed point (valid: fast path guards x in [0,1];
    # the 1/65535 scale is folded into the L0 coefficient sets on host)
    x_d = nc.dram_tensor("x", [B, C, H, W], u16, kind="ExternalInput")
    sel0_d = nc.dram_tensor("sel0", [8, 75, 128], u8,
                            kind="ExternalInput").ap()
    cof_d = nc.dram_tensor("coefs", [11, 128, 4], f32, kind="ExternalInput").ap()
    # y quantized to u8: y in [0,1] exactly (all 16 gates map [0,1]^2 into
    # [0,1] and the mixes are convex); 254*y + 0.5 is folded into the L6
    # coefficient set on host, decode is (q - 0.5)/254.
    y_d = nc.dram_tensor("y", [128, P], u8, kind="ExternalOutput").ap()
    sels_c = nc.inline_tensor(
        np.ascontiguousarray(build_sel_mats().reshape(24, 128, 128)),
        name="selsc").ap()
    # one-hot scatter mats: src channel c -> window row c*25 + o, o=(dh,dw)
    shc = np.zeros((25, 3, 75), dtype=np.float32)
    for o in range(25):
        for c in range(C):
            shc[o, c, c * 25 + o] = 1.0
    shc_c = nc.inline_tensor(
        np.ascontiguousarray(np.transpose(shc, (1, 0, 2)).reshape(3, 25 * 75)),
        name="shiftc").ap()

    with TileContext(nc) as tc:
        with (
            tc.tile_pool(name="const", bufs=1) as cpool,
            tc.tile_pool(name="ab", bufs=3) as ab,
            tc.tile_pool(name="uvw", bufs=4) as uvw,
            tc.tile_pool(name="s0", bufs=8) as s0p,
            tc.tile_pool(name="s1", bufs=4) as s1p,
            tc.tile_pool(name="s2", bufs=4) as s2p,
            tc.tile_pool(name="s3", bufs=4) as s3p,
            tc.tile_pool(name="s45", bufs=4) as s45p,
            tc.tile_pool(name="yo", bufs=1) as yop,
            tc.tile_pool(name="abw", bufs=3) as abw,
            tc.tile_pool(name="ps", bufs=2, space="PSUM") as ps,
        ):
            sel_t = []
            for m in range(24):
                t = cpool.tile([128, 128], f32, tag=f"sel{m}")
                nc.sync.dma_start(t[:], sels_c[m])
                sel_t.append(t)
            sel0_t = []
            for m in range(8):
                tu = cpool.tile([75, 128], u8, tag=f"sel0u_{m}")
                nc.sync.dma_start(tu[:], sel0_d[m])
                t = cpool.tile([75, 128], f32, tag=f"sel0_{m}")
                nc.vector.tensor_copy(t[:], tu[:])
                sel0_t.append(t)
            cof_t = []
            for m in range(11):
                t = cpool.tile([128, 4], f32, tag=f"cof{m}")
                nc.sync.dma_start(t[:], cof_d[m])
                cof_t.append(t)

            shc_t = cpool.tile([3, 25 * 75], f32, tag="shc")
            nc.sync.dma_start(shc_t[:], shc_c)

            def sel(l, side, rel):
                return sel_t[(l - 1) * 4 + side * 2 + rel]

            def level_core(A_ap, B_ap, cs, out_tile):
                """u,v,w,out from A/B access patterns + coef tile."""
                u = uvw.tile([128, P], f32, tag="u")
                v = uvw.tile([128, P], f32, tag="v")
                w = uvw.tile([128, P], f32, tag="w")
                nc.scalar.activation(u[:], A_ap, Ident,
                                     bias=cs[:, 2:3], scale=cs[:, 3:4])
                nc.scalar.activation(v[:], A_ap, Ident,
                                     bias=cs[:, 0:1], scale=cs[:, 1:2])
                nc.vector.tensor_mul(w[:], u[:], B_ap)
                nc.vector.tensor_add(out_tile[:], w[:], v[:])

            def level_mm(l, src0, src1, cs, out_tile):
                pA = ps.tile([128, P], f32, tag="pA")
                pB = ps.tile([128, P], f32, tag="pB")
                for (c0, c1) in COLS:
                    for rel, src in ((0, src0), (1, src1)):
                        nc.tensor.matmul(pA[:, c0:c1], sel(l, 0, rel)[:],
                                         src[:, c0:c1],
                                         start=(rel == 0), stop=(rel == 1))
                        nc.tensor.matmul(pB[:, c0:c1], sel(l, 1, rel)[:],
                                         src[:, c0:c1],
                                         start=(rel == 0), stop=(rel == 1))
                level_core(pA[:], pB[:], cs, out_tile)

            s2t = [None] * B
            s3t = [None] * 8
            s4t = [None] * 4
            s5t = [None] * 2
            for b in range(B):
                # wx[(c,dh,dw), (hp,wp)] = x[b, c, dh+hp, dw+wp]:
                # contiguous DMA of x[b], DVE dequantize, 25 strided window
                # copies, one-hot scatter matmuls into the 75 window rows.
                xb_u = ab.tile([C, H * W], u16, tag="xbu")
                nc.sync.dma_start(xb_u[:],
                                  AP(x_d, b * C * H * W,
                                     [[H * W, C], [1, H * W]]))
                xb_f = ab.tile([C, H * W], f32, tag="xbf")
                nc.vector.tensor_copy(xb_f[:], xb_u[:])
                xbv = xb_f[:]
                xb_pitch = xbv.ap[0][0]
                wxp = ps.tile([128, P], f32, tag="pA")
                for o in range(25):
                    dh, dw = o // RF, o % RF
                    xw = abw.tile([C, P], f32, tag="xw")
                    src = AP(xbv.tensor, xbv.offset + dh * W + dw,
                             [[xb_pitch, C], [W, 28], [1, 28]])
                    nc.vector.tensor_copy(xw[:], src)
                    for (c0, c1) in COLS:
                        nc.tensor.matmul(wxp[0:75, c0:c1],
                                         shc_t[:, o * 75:(o + 1) * 75],
                                         xw[:, c0:c1],
                                         start=(o == 0), stop=(o == 24))
                wx = ab.tile([75, P], f32, tag="wx")
                nc.scalar.copy(wx[:], wxp[0:75, :])

                s0t = []
                for g in range(4):
                    pA = ps.tile([128, P], f32, tag="pA")
                    pB = ps.tile([128, P], f32, tag="pB")
                    for (c0, c1) in COLS:
                        for side, pt in ((0, pA), (1, pB)):
                            nc.tensor.matmul(pt[:, c0:c1],
                                             sel0_t[2 * g + side][:],
                                             wx[:, c0:c1],
                                             start=True, stop=True)
                    st = s0p.tile([128, P], f32, tag="s0")
                    level_core(pA[:], pB[:], cof_t[g], st)
                    s0t.append(st)
                s1t = []
                for d in range(2):
                    st = s1p.tile([128, P], f32, tag="s1")
                    level_mm(1, s0t[2 * d], s0t[2 * d + 1], cof_t[4 + d], st)
                    s1t.append(st)
                st = s2p.tile([128, P], f32, tag="s2")
                level_mm(2, s1t[0], s1t[1], cof_t[6], st)
                s2t[b] = st
                if b % 2 == 1:
                    g3 = b // 2
                    st = s3p.tile([128, P], f32, tag="s3")
                    level_mm(3, s2t[b - 1], s2t[b], cof_t[7], st)
                    s3t[g3] = st
                if b % 4 == 3:
                    g4 = b // 4
                    st = s45p.tile([128, P], f32, tag="s4")
                    level_mm(4, s3t[2 * g4], s3t[2 * g4 + 1], cof_t[8], st)
                    s4t[g4] = st
                if b % 8 == 7:
                    g5 = b // 8
                    st = s45p.tile([128, P], f32, tag="s5")
                    level_mm(5, s4t[2 * g5], s4t[2 * g5 + 1], cof_t[9], st)
                    s5t[g5] = st
            yf = s45p.tile([128, P], f32, tag="s6")
            level_mm(6, s5t[0], s5t[1], cof_t[10], yf)
            yt = yop.tile([128, P], u8, tag="yq")
            nc.vector.tensor_copy(yt[:], yf[:])
            nc.sync.dma_start(y_d[:], yt[:])
    nc.compile()
    return nc


class _FastRunner:
    """Builds the structured-path program once and keeps a jitted
    shard_map(bass_exec) callable so repeat calls skip retracing."""

    def __init__(self):
        import jax
        import concourse.mybir as mybir
        from jax.sharding import Mesh, PartitionSpec
        from concourse.bass2jax import (
            _bass_exec_p, partition_id_tensor, install_neuronx_cc_hook)
        import warnings
        with warnings.catch_warnings():
            warnings.simplefilter("ignore")
            try:
                from jax.experimental.shard_map import shard_map
            except ImportError:
                from jax import shard_map

        install_neuronx_cc_hook()
        nc = _build_bass_fast()
        self.nc = nc
        partition_name = (nc.partition_id_tensor.name
                          if nc.partition_id_tensor else None)
        in_names, out_names, out_avals, zero_outs = [], [], [], []
        for alloc in nc.m.functions[0].allocations:
            if not isinstance(alloc, mybir.MemoryLocationSet):
                continue
            name = alloc.memorylocations[0].name
            if alloc.kind == "ExternalInput":
                if name != partition_name:
                    in_names.append(name)
            elif alloc.kind == "ExternalOutput":
                out_names.append(name)
                shape = tuple(alloc.tensor_shape)
                dtype = mybir.dt.np(alloc.dtype)
                out_avals.append(jax.core.ShapedArray(shape, dtype))
                zero_outs.append((shape, dtype))
        self.in_names = in_names
        self.out_names = out_names
        self.zero_outs = zero_outs
        n_params = len(in_names)
        n_outs = len(out_names)
        bind_names = tuple(in_names + out_names
                           + ([partition_name] if partition_name else []))

        def _body(*args):
            operands = list(args)
            if partition_name is not None:
                operands.append(partition_id_tensor())
            return tuple(_bass_exec_p.bind(
                *operands, out_avals=tuple(out_avals), in_names=bind_names,
                out_names=tuple(out_names),
                lowering_input_output_aliases=(),
                sim_require_finite=True, sim_require_nnan=True, nc=nc))

        devices = jax.devices()[:NCORES]
        assert len(devices) == NCORES
        mesh = Mesh(np.asarray(devices), ("core",))
        self.sharded = jax.jit(
            shard_map(_body, mesh=mesh,
                      in_specs=(PartitionSpec("core"),) * (n_params + n_outs),
                      out_specs=(PartitionSpec("core"),) * n_outs,
                      check_rep=False),
            donate_argnums=tuple(range(n_params, n_params + n_outs)),
            keep_unused=True)

    def __call__(self, arrays_by_name):
        args = [arrays_by_name[n] for n in self.in_names]
        args += [np.zeros((NCORES * s[0], *s[1:]), d)
                 for (s, d) in self.zero_outs]
        outs = self.sharded(*args)
        # asarray immediately after async dispatch: the d2h request
        # overlaps the dispatch round trip.
        return {n: np.asarray(o) for n, o in zip(self.out_names, outs)}


_COF_MEMO = {}
_SEL0_MEMO = {}


def _cof_folded(ws):
    key = b"".join(np.ascontiguousarray(w).tobytes() for w in ws)
    hit = _COF_MEMO.get("cof")
    if hit is not None and hit[0] == key:
        return hit[1]
    coefs = [_coeffs(w) for w in ws]
    cof = np.stack([build_coef_sets(coefs, c) for c in range(NCORES)])
    # fold the u16 leaf dequantization into the L0 sets (a = q * s):
    s = np.float32(1.0 / 65535.0)
    cof[:, 0:4, :, 1] *= s          # c1 * s
    cof[:, 0:4, :, 2] *= s          # c2 * s
    cof[:, 0:4, :, 3] *= s * s      # c3 * s^2
    # fold the u8 output quantization (254*y + 0.5) into the L6 set:
    cof[:, 10] *= np.float32(254.0)
    cof[:, 10, :, 0] += np.float32(0.5)
    cof = np.ascontiguousarray(cof.reshape(NCORES * 11, 128, 4))
    _COF_MEMO["cof"] = (key, cof)
    return cof


def _kernel_fast(x, ws, widx):
    if "fast" not in _CACHE:
        _CACHE["fast"] = _FastRunner()
    runner = _CACHE["fast"]

    cof = _cof_folded(ws)
    skey = (id(widx[0]), id(widx[1]))
    sel0 = _SEL0_MEMO.get(skey)
    if sel0 is None:
        sel0 = build_sel0_all(widx).reshape(NCORES * 8, 75, 128)
        _SEL0_MEMO.clear()
        _SEL0_MEMO[skey] = sel0
    xq = (x * np.float32(65535.0) + np.float32(0.5)).astype(np.uint16)
    xg = np.ascontiguousarray(
        np.broadcast_to(xq, (NCORES,) + xq.shape)).reshape(NCORES * B, C, H, W)

    res = runner({"x": xg, "sel0": sel0, "coefs": cof})
    q = res["y"].astype(np.float32)                       # [8*128, 784]
    y = (q - np.float32(0.5)) * np.float32(1.0 / 254.0)
    y = y.reshape(NCORES, B, KLOC, P).transpose(1, 0, 2, 3)
    return np.ascontiguousarray(y.reshape(B, K, P, 1))


# ------------------------------------------------------------- fallback ----

def gather_leaves(x, left_idx, right_idx):
    """Host leaf gather with jax clamp semantics.

    Returns A, B: [NCORES, B, 4, 128, P] float32 where partition row of tile g
    is (k2=row//64 within pair {2g,2g+1}, s=row%64).
    """
    xf = np.ascontiguousarray(x).reshape(B, C * H * W)
    outs = []
    for idx in (left_idx, right_idx):
        h = np.clip(idx[..., 0], 0, H - 1).astype(np.int64)
        w = np.clip(idx[..., 1], 0, W - 1).astype(np.int64)
        c = np.clip(idx[..., 2], 0, C - 1).astype(np.int64)
        flat = c * (H * W) + h * W + w          # [K, P, S]
        flat = np.transpose(flat, (0, 2, 1))     # [K, S, P]
        g = xf[:, flat]                          # [B, K, S, P]
        g = g.reshape(B, NCORES, KLOC, S, P)
        g = np.transpose(g, (1, 0, 2, 3, 4))     # [cores, B, KLOC, S, P]
        outs.append(np.ascontiguousarray(
            g.reshape(NCORES, B, 4, 128, P).astype(np.float32)))
    return outs


def _build_bass_fallback():
    import concourse.mybir as mybir
    from concourse import bacc
    from concourse.tile import TileContext

    f32 = mybir.dt.float32
    Ident = mybir.ActivationFunctionType.Identity

    nc = bacc.Bacc("TRN2", target_bir_lowering=False, debug=False,
                   num_devices=NCORES)
    Ain_d = nc.dram_tensor("Ain", [B, 4, 128, P], f32,
                           kind="ExternalInput").ap()
    Bin_d = nc.dram_tensor("Bin", [B, 4, 128, P], f32,
                           kind="ExternalInput").ap()
    sel_d = nc.dram_tensor("sels", [24, 128, 128], f32, kind="ExternalInput").ap()
    cof_d = nc.dram_tensor("coefs", [11, 128, 4], f32, kind="ExternalInput").ap()
    y_d = nc.dram_tensor("y", [128, P], f32, kind="ExternalOutput").ap()

    with TileContext(nc) as tc:
        with (
            tc.tile_pool(name="const", bufs=1) as cpool,
            tc.tile_pool(name="ab", bufs=4) as ab,
            tc.tile_pool(name="uvw", bufs=4) as uvw,
            tc.tile_pool(name="s0", bufs=8) as s0p,
            tc.tile_pool(name="s1", bufs=4) as s1p,
            tc.tile_pool(name="s2", bufs=4) as s2p,
            tc.tile_pool(name="s3", bufs=4) as s3p,
            tc.tile_pool(name="s45", bufs=4) as s45p,
            tc.tile_pool(name="ps", bufs=2, space="PSUM") as ps,
        ):
            sel_t = []
            for m in range(24):
                t = cpool.tile([128, 128], f32, tag=f"sel{m}")
                nc.sync.dma_start(t[:], sel_d[m])
                sel_t.append(t)
            cof_t = []
            for m in range(11):
                t = cpool.tile([128, 4], f32, tag=f"cof{m}")
                nc.sync.dma_start(t[:], cof_d[m])
                cof_t.append(t)

            def sel(l, side, rel):
                return sel_t[(l - 1) * 4 + side * 2 + rel]

            def level_core(A_ap, B_ap, cs, out_tile):
                u = uvw.tile([128, P], f32, tag="u")
                v = uvw.tile([128, P], f32, tag="v")
                w = uvw.tile([128, P], f32, tag="w")
                nc.scalar.activation(u[:], A_ap, Ident,
                                     bias=cs[:, 2:3], scale=cs[:, 3:4])
                nc.scalar.activation(v[:], A_ap, Ident,
                                     bias=cs[:, 0:1], scale=cs[:, 1:2])
                nc.vector.tensor_mul(w[:], u[:], B_ap)
                nc.vector.tensor_add(out_tile[:], w[:], v[:])

            def level_mm(l, src0, src1, cs, out_tile):
                pA = ps.tile([128, P], f32, tag="pA")
                pB = ps.tile([128, P], f32, tag="pB")
                for (c0, c1) in COLS:
                    for rel, src in ((0, src0), (1, src1)):
                        nc.tensor.matmul(pA[:, c0:c1], sel(l, 0, rel)[:],
                                         src[:, c0:c1],
                                         start=(rel == 0), stop=(rel == 1))
                        nc.tensor.matmul(pB[:, c0:c1], sel(l, 1, rel)[:],
                                         src[:, c0:c1],
                                         start=(rel == 0), stop=(rel == 1))
                level_core(pA[:], pB[:], cs, out_tile)

            s2t = [None] * B
            s3t = [None] * 8
            s4t = [None] * 4
            s5t = [None] * 2
            for b in range(B):
                s0t = []
                for g in range(4):
                    At = ab.tile([128, P], f32, tag="Ain")
                    Bt = ab.tile([128, P], f32, tag="Bin")
                    nc.sync.dma_start(At[:], Ain_d[b, g])
                    nc.sync.dma_start(Bt[:], Bin_d[b, g])
                    st = s0p.tile([128, P], f32, tag="s0")
                    level_core(At[:], Bt[:], cof_t[g], st)
                    s0t.append(st)
                s1t = []
                for d in range(2):
                    st = s1p.tile([128, P], f32, tag="s1")
                    level_mm(1, s0t[2 * d], s0t[2 * d + 1], cof_t[4 + d], st)
                    s1t.append(st)
                st = s2p.tile([128, P], f32, tag="s2")
                level_mm(2, s1t[0], s1t[1], cof_t[6], st)
                s2t[b] = st
                if b % 2 == 1:
                    g3 = b // 2
                    st = s3p.tile([128, P], f32, tag="s3")
                    level_mm(3, s2t[b - 1], s2t[b], cof_t[7], st)
                    s3t[g3] = st
                if b % 4 == 3:
                    g4 = b // 4
                    st = s45p.tile([128, P], f32, tag="s4")
                    level_mm(4, s3t[2 * g4], s3t[2 * g4 + 1], cof_t[8], st)
                    s4t[g4] = st
                if b % 8 == 7:
                    g5 = b // 8
                    st = s45p.tile([128, P], f32, tag="s5")
                    level_mm(5, s4t[2 * g5], s4t[2 * g5 + 1], cof_t[9], st)
                    s5t[g5] = st
            yt = s45p.tile([128, P], f32, tag="s6")
            level_mm(6, s5t[0], s5t[1], cof_t[10], yt)
            nc.sync.dma_start(y_d[:], yt[:])
    nc.compile()
    return nc


def _kernel_fallback(x, ws, left_idx, right_idx):
    from concourse.bass_utils import run_bass_kernel_spmd

    if "fallback" not in _CACHE:
        _CACHE["fallback"] = _build_bass_fallback()
    nc = _CACHE["fallback"]

    coefs = [_coeffs(w) for w in ws]
    sels = build_sel_mats().reshape(24, 128, 128)
    csets = [build_coef_sets(coefs, c) for c in range(NCORES)]
    Ain, Bin = gather_leaves(x, left_idx, right_idx)
    in_maps = [
        {"Ain": Ain[c], "Bin": Bin[c], "sels": sels, "coefs": csets[c]}
        for c in range(NCORES)
    ]
    res = run_bass_kernel_spmd(nc, in_maps, core_ids=list(range(NCORES)))
    out = np.zeros((B, K, P, 1), dtype=np.float32)
    for c in range(NCORES):
        yc = res.results[c]["y"].reshape(B, KLOC, P)
        out[:, c * KLOC:(c + 1) * KLOC, :, 0] = yc
    return out


def kernel(x, w0, w1, w2, w3, w4, w5, w6, left_idx, right_idx):
    x = np.asarray(x, dtype=np.float32)
    ws = [np.asarray(w, dtype=np.float32) for w in (w0, w1, w2, w3, w4, w5, w6)]
    left_idx = np.asarray(left_idx)
    right_idx = np.asarray(right_idx)

    widx = detect_structure_memo(left_idx, right_idx)
    if widx is not None and x.min() >= 0.0 and x.max() <= 1.0:
        return _kernel_fast(x, ws, widx)
    return _kernel_fallback(x, ws, left_idx, right_idx)


# revision 37
# speedup vs baseline: 1.1000x; 1.1000x over previous
"""Trainium2 Bass kernel for nn_LogicConv3d (DiffLogic conv tree).

Per-call wall clock is dominated by the axon tunnel (~70ms fixed RTT +
~25-55MB/s for incompressible bytes), so the design minimizes wire bytes
and round trips; device compute (~1.5ms) is irrelevant by comparison.

Fast path (structured conv indices AND x in [0,1]):
  - Shard num_kernels K=64 across 8 cores (8 kernels/core).
  - Uploads per call: x as u16 fixed point (98KB/core; the 1/65535
    dequant scale is folded into the L0 coefficient sets), per-core u8
    one-hot L0 gather matrices (zstd on the wire crushes one-hot u8),
    per-core f32 coefficient sets (22KB), u8 zero output buffers
    (compress to ~nothing). The 24 level-shuffle matrices and the 25
    channel->window-row scatter matrices are baked into the NEFF as
    constants (inline_tensor).
  - Device, per batch: contiguous DMA of x[b] ([3,1024] u16), DVE
    dequantize to f32, 25 strided window copies ([3,784] views at shift
    (dh,dw)), one-hot scatter matmuls accumulate them into the [75,784]
    im2col window tile in PSUM. (Overlapping-stride DMA descriptors
    corrupt on this HW for most src offsets -- engine ops only.)
    Then 7 tree levels:
      A,B = PE one-hot selection matmuls (even/odd child shuffle)
      u = c3*A + c2; v = c1*A + c0 (ScalarE), state = u*B + v (VectorE)
    Deep levels (3-6) pack batches into partitions to keep lanes full.
  - Output: [128=(b16,k8), 784] u8 per core; 254*y + 0.5 is folded into
    the L6 coefficient set (y in [0,1] exactly: all 16 gates map [0,1]^2
    into [0,1] and softmax mixes are convex); host decodes (q-0.5)/254.
    Max abs error ~2e-3 vs the 2e-2 gate.
  - Dispatch through a cached jax.jit(shard_map(bass_exec)) callable so
    repeat calls skip retracing; outputs fetched immediately after the
    async dispatch so the d2h request overlaps the dispatch round trip.
  - Host preprocessing is memoized: the full structure check runs once
    per distinct index-buffer pair, coefficient folding once per weight
    bytes.

Fallback (arbitrary indices or x outside [0,1]): host gather + the
original full-precision Ain/Bin program via run_bass_kernel_spmd.
"""

import numpy as np

B, C, H, W = 16, 3, 32, 32
K = 64
RF = 5
DEPTH = 6
S = 2 ** DEPTH          # 64
P = 784                 # 28*28 conv positions
NCORES = 8
KLOC = K // NCORES      # 8 kernels per core
COLS = [(0, 512), (512, 784)]   # fp32 matmul moving-dim <= 512
XLEN = 27 * 32 + 27 + 1         # 892: max in-window flat offset + 1

_GATE_COEFFS = np.array([
    [0, 0, 0, 0], [0, 0, 0, 1], [0, 1, 0, -1], [0, 1, 0, 0],
    [0, 0, 1, -1], [0, 0, 1, 0], [0, 1, 1, -2], [0, 1, 1, -1],
    [1, -1, -1, 1], [1, -1, -1, 2], [1, 0, -1, 0], [1, 0, -1, 1],
    [1, -1, 0, 0], [1, -1, 0, 1], [1, 0, 0, -1], [1, 0, 0, 0],
], dtype=np.float32)


def _softmax(x, axis=-1):
    x = x - x.max(axis=axis, keepdims=True)
    e = np.exp(x)
    return e / e.sum(axis=axis, keepdims=True)


def _coeffs(w):
    """w: [S_l, K, 16] -> [S_l, K, 4] polynomial coefficients."""
    return _softmax(w.astype(np.float64)).astype(np.float32) @ _GATE_COEFFS


def build_sel_mats():
    """24 one-hot matrices [6 levels][side 2][rel 2][128 rows(src), 128 cols(dst)].

    Level l in 1..6 consumes state_{l-1}; dst tile column j maps to a source
    row in one of two source tile instances (rel 0/1). Patterns are shared
    across batches / dst-tile instances by construction.
    """
    mats = np.zeros((6, 2, 2, 128, 128), dtype=np.float32)

    def put(l, rel, row, j):
        mats[l - 1, 0, rel, row, j] = 1.0      # A side (even child)
        mats[l - 1, 1, rel, row + 1, j] = 1.0  # B side (odd child = row+1)

    for j in range(128):
        # L1: dst id=128d+j = kloc*32+t, kloc=4d+j//32 ; src id = kloc*64+2t
        k, t = j // 32, j % 32
        put(1, k // 2, (k % 2) * 64 + 2 * t, j)
        # L2: kloc=j//16, t=j%16 ; src id = kloc*32+2t (256 nodes, 2 tiles)
        k, t = j // 16, j % 16
        put(2, k // 4, (k % 4) * 32 + 2 * t, j)
        # L3: dst (bhat=j//64, id=j%64=k*8+t); src = per-batch state2[bhat]
        bh, idd = j // 64, j % 64
        k, t = idd // 8, idd % 8
        put(3, bh, k * 16 + 2 * t, j)
        # L4: dst (bhat=j//32, id=k*4+t); src state3 packed nb=2
        bh, idd = j // 32, j % 32
        k, t = idd // 4, idd % 4
        put(4, bh // 2, (bh % 2) * 64 + k * 8 + 2 * t, j)
        # L5: dst (bhat=j//16, id=k*2+t); src state4 packed nb=4
        bh, idd = j // 16, j % 16
        k, t = idd // 2, idd % 2
        put(5, bh // 4, (bh % 4) * 32 + k * 4 + 2 * t, j)
        # L6: dst (bhat=j//8, k=j%8); src state5 packed nb=8
        bh, k = j // 8, j % 8
        put(6, bh // 8, (bh % 8) * 16 + k * 2, j)
    return mats


def build_coef_sets(coefs, core):
    """11 coefficient sets [128, 4] for one core (kernels core*8..core*8+7).

    Sets: 0-3 L0 tiles g0..g3; 4-5 L1 d0,d1; 6 L2; 7-10 L3..L6.
    coefs: list of 7 arrays [S_l, K, 4].
    """
    k0 = core * KLOC
    out = np.zeros((11, 128, 4), dtype=np.float32)
    r = np.arange(128)
    for g in range(4):
        out[g] = coefs[0][r % 64, k0 + 2 * g + r // 64]
    for d in range(2):
        out[4 + d] = coefs[1][r % 32, k0 + 4 * d + r // 32]
    out[6] = coefs[2][r % 16, k0 + r // 16]
    out[7] = coefs[3][(r % 64) % 8, k0 + (r % 64) // 8]
    out[8] = coefs[4][(r % 32) % 4, k0 + (r % 32) // 4]
    out[9] = coefs[5][(r % 16) % 2, k0 + (r % 16) // 2]
    out[10] = coefs[6][0, k0 + r % 8]
    return out


def detect_structure(left_idx, right_idx):
    """If idx[k,p,s] = window_base[k,s] + conv_offset[p] (as produced by the
    reference's setup_inputs), return (widxL, widxR): [K, S] window ids in
    [0, 75) = (c*5+dh)*5+dw. Else None."""
    poff = ((np.arange(28, dtype=np.int32)[:, None] * W
             + np.arange(28, dtype=np.int32)[None, :]).ravel())
    ph, pw = poff // W, poff % W                          # [P]
    pvec = np.stack([ph, pw, np.zeros_like(ph)], axis=-1)  # [P, 3]
    out = []
    for idx in (left_idx, right_idx):
        idx = idx.astype(np.int32, copy=False)
        base = idx[:, 0, :, :]                            # [K, S, 3] (p=0)
        hb, wb, cb = base[..., 0], base[..., 1], base[..., 2]
        if (base.min() < 0 or hb.max() >= RF or wb.max() >= RF
                or cb.max() >= C):
            return None
        if not np.array_equal(
                idx, base[:, None, :, :] + pvec[None, :, None, :]):
            return None
        out.append((cb * RF * RF + hb * RF + wb).astype(np.int64))  # [K, S]
    return out


_IDX_MEMO = {}


def detect_structure_memo(left_idx, right_idx):
    """Memoized structure check. Keyed on array identity plus a strided
    content sample; full check on first sight of a buffer pair."""
    key = (id(left_idx), id(right_idx), left_idx.shape, right_idx.shape)
    samp = (left_idx[::13, ::17, ::7].tobytes(),
            right_idx[::13, ::17, ::7].tobytes())
    hit = _IDX_MEMO.get(key)
    if hit is not None and hit[0] == samp:
        return hit[1]
    widx = detect_structure(left_idx, right_idx)
    _IDX_MEMO[key] = (samp, widx)
    return widx


def build_sel0_all(widx):
    """[NCORES, 8, 75, 128] u8 one-hot L0 gather matrices, all cores.

    mat[c, g*2+side][row=window id, col=(k2=j//64, s=j%64)] selects the
    leaf window for kernel c*8+2g+(j//64), leaf s."""
    widxL, widxR = widx
    out = np.zeros((NCORES, 8, 75, 128), dtype=np.uint8)
    j = np.arange(128)
    for c in range(NCORES):
        for g in range(4):
            kg = c * KLOC + 2 * g + j // 64
            out[c, 2 * g, widxL[kg, j % 64], j] = 1
            out[c, 2 * g + 1, widxR[kg, j % 64], j] = 1
    return out


# ---------------------------------------------------------------- device ----

_CACHE = {}


def _build_bass_fast():
    """Structured-path program: consumes x directly, builds windows on
    device, level-shuffle matrices baked in as NEFF constants."""
    import concourse.mybir as mybir
    from concourse import bacc
    from concourse.tile import TileContext
    from bass_rust import AP

    f32 = mybir.dt.float32
    u8 = mybir.dt.uint8
    u16 = mybir.dt.uint16
    Ident = mybir.ActivationFunctionType.Identity

    nc = bacc.Bacc("TRN2", target_bir_lowering=False, debug=False,
                   num_devices=NCORES)
    # x quantized to u16 fixed point (valid: fast path guards x in [0,1];
    # the 1/65535 scale is folded into the L0 coefficient sets on host)
    x_d = nc.dram_tensor("x", [B, C, H, W], u16, kind="ExternalInput")
    sel0_d = nc.dram_tensor("sel0", [8, 75, 128], u8,
                            kind="ExternalInput").ap()
    cof_d = nc.dram_tensor("coefs", [11, 128, 4], f32, kind="ExternalInput").ap()
    # y quantized to u8: y in [0,1] exactly (all 16 gates map [0,1]^2 into
    # [0,1] and the mixes are convex); 254*y + 0.5 is folded into the L6
    # coefficient set on host, decode is (q - 0.5)/254.
    y_d = nc.dram_tensor("y", [128, P], u8, kind="ExternalOutput").ap()
    sels_c = nc.inline_tensor(
        np.ascontiguousarray(build_sel_mats().reshape(24, 128, 128)),
        name="selsc").ap()
    # one-hot scatter mats: src channel c -> window row c*25 + o, o=(dh,dw)
    shc = np.zeros((25, 3, 75), dtype=np.float32)
    for o in range(25):
        for c in range(C):
            shc[o, c, c * 25 + o] = 1.0
    shc_c = nc.inline_tensor(
        np.ascontiguousarray(np.transpose(shc, (1, 0, 2)).reshape(3, 25 * 75)),
        name="shiftc").ap()

    with TileContext(nc) as tc:
        with (
            tc.tile_pool(name="const", bufs=1) as cpool,
            tc.tile_pool(name="ab", bufs=3) as ab,
            tc.tile_pool(name="uvw", bufs=4) as uvw,
            tc.tile_pool(name="s0", bufs=8) as s0p,
            tc.tile_pool(name="s1", bufs=4) as s1p,
            tc.tile_pool(name="s2", bufs=4) as s2p,
            tc.tile_pool(name="s3", bufs=4) as s3p,
            tc.tile_pool(name="s45", bufs=4) as s45p,
            tc.tile_pool(name="yo", bufs=1) as yop,
            tc.tile_pool(name="abw", bufs=3) as abw,
            tc.tile_pool(name="ps", bufs=2, space="PSUM") as ps,
        ):
            sel_t = []
            for m in range(24):
                t = cpool.tile([128, 128], f32, tag=f"sel{m}")
                nc.sync.dma_start(t[:], sels_c[m])
                sel_t.append(t)
            sel0_t = []
            for m in range(8):
                tu = cpool.tile([75, 128], u8, tag=f"sel0u_{m}")
                nc.sync.dma_start(tu[:], sel0_d[m])
                t = cpool.tile([75, 128], f32, tag=f"sel0_{m}")
                nc.vector.tensor_copy(t[:], tu[:])
                sel0_t.append(t)
            cof_t = []
            for m in range(11):
                t = cpool.tile([128, 4], f32, tag=f"cof{m}")
                nc.sync.dma_start(t[:], cof_d[m])
                cof_t.append(t)

            shc_t = cpool.tile([3, 25 * 75], f32, tag="shc")
            nc.sync.dma_start(shc_t[:], shc_c)

            def sel(l, side, rel):
                return sel_t[(l - 1) * 4 + side * 2 + rel]

            def level_core(A_ap, B_ap, cs, out_tile):
                """u,v,w,out from A/B access patterns + coef tile."""
                u = uvw.tile([128, P], f32, tag="u")
                v = uvw.tile([128, P], f32, tag="v")
                w = uvw.tile([128, P], f32, tag="w")
                nc.scalar.activation(u[:], A_ap, Ident,
                                     bias=cs[:, 2:3], scale=cs[:, 3:4])
                nc.scalar.activation(v[:], A_ap, Ident,
                                     bias=cs[:, 0:1], scale=cs[:, 1:2])
                nc.vector.tensor_mul(w[:], u[:], B_ap)
                nc.vector.tensor_add(out_tile[:], w[:], v[:])

            def level_mm(l, src0, src1, cs, out_tile):
                pA = ps.tile([128, P], f32, tag="pA")
                pB = ps.tile([128, P], f32, tag="pB")
                for (c0, c1) in COLS:
                    for rel, src in ((0, src0), (1, src1)):
                        nc.tensor.matmul(pA[:, c0:c1], sel(l, 0, rel)[:],
                                         src[:, c0:c1],
                                         start=(rel == 0), stop=(rel == 1))
                        nc.tensor.matmul(pB[:, c0:c1], sel(l, 1, rel)[:],
                                         src[:, c0:c1],
                                         start=(rel == 0), stop=(rel == 1))
                level_core(pA[:], pB[:], cs, out_tile)

            s2t = [None] * B
            s3t = [None] * 8
            s4t = [None] * 4
            s5t = [None] * 2
            for b in range(B):
                # wx[(c,dh,dw), (hp,wp)] = x[b, c, dh+hp, dw+wp]:
                # contiguous DMA of x[b], DVE dequantize, 25 strided window
                # copies, one-hot scatter matmuls into the 75 window rows.
                xb_u = ab.tile([C, H * W], u16, tag="xbu")
                nc.sync.dma_start(xb_u[:],
                                  AP(x_d, b * C * H * W,
                                     [[H * W, C], [1, H * W]]))
                xb_f = ab.tile([C, H * W], f32, tag="xbf")
                nc.vector.tensor_copy(xb_f[:], xb_u[:])
                xbv = xb_f[:]
                xb_pitch = xbv.ap[0][0]
                wxp = ps.tile([128, P], f32, tag="pA")
                for o in range(25):
                    dh, dw = o // RF, o % RF
                    xw = abw.tile([C, P], f32, tag="xw")
                    src = AP(xbv.tensor, xbv.offset + dh * W + dw,
                             [[xb_pitch, C], [W, 28], [1, 28]])
                    nc.vector.tensor_copy(xw[:], src)
                    for (c0, c1) in COLS:
                        nc.tensor.matmul(wxp[0:75, c0:c1],
                                         shc_t[:, o * 75:(o + 1) * 75],
                                         xw[:, c0:c1],
                                         start=(o == 0), stop=(o == 24))
                wx = ab.tile([75, P], f32, tag="wx")
                nc.scalar.copy(wx[:], wxp[0:75, :])

                s0t = []
                for g in range(4):
                    pA = ps.tile([128, P], f32, tag="pA")
                    pB = ps.tile([128, P], f32, tag="pB")
                    for (c0, c1) in COLS:
                        for side, pt in ((0, pA), (1, pB)):
                            nc.tensor.matmul(pt[:, c0:c1],
                                             sel0_t[2 * g + side][:],
                                             wx[:, c0:c1],
                                             start=True, stop=True)
                    st = s0p.tile([128, P], f32, tag="s0")
                    level_core(pA[:], pB[:], cof_t[g], st)
                    s0t.append(st)
                s1t = []
                for d in range(2):
                    st = s1p.tile([128, P], f32, tag="s1")
                    level_mm(1, s0t[2 * d], s0t[2 * d + 1], cof_t[4 + d], st)
                    s1t.append(st)
                st = s2p.tile([128, P], f32, tag="s2")
                level_mm(2, s1t[0], s1t[1], cof_t[6], st)
                s2t[b] = st
                if b % 2 == 1:
                    g3 = b // 2
                    st = s3p.tile([128, P], f32, tag="s3")
                    level_mm(3, s2t[b - 1], s2t[b], cof_t[7], st)
                    s3t[g3] = st
                if b % 4 == 3:
                    g4 = b // 4
                    st = s45p.tile([128, P], f32, tag="s4")
                    level_mm(4, s3t[2 * g4], s3t[2 * g4 + 1], cof_t[8], st)
                    s4t[g4] = st
                if b % 8 == 7:
                    g5 = b // 8
                    st = s45p.tile([128, P], f32, tag="s5")
                    level_mm(5, s4t[2 * g5], s4t[2 * g5 + 1], cof_t[9], st)
                    s5t[g5] = st
            yf = s45p.tile([128, P], f32, tag="s6")
            level_mm(6, s5t[0], s5t[1], cof_t[10], yf)
            yt = yop.tile([128, P], u8, tag="yq")
            nc.vector.tensor_copy(yt[:], yf[:])
            nc.sync.dma_start(y_d[:], yt[:])
    nc.compile()
    return nc


class _FastRunner:
    """Builds the structured-path program once and keeps a jitted
    shard_map(bass_exec) callable so repeat calls skip retracing."""

    def __init__(self):
        import jax
        import concourse.mybir as mybir
        from jax.sharding import Mesh, PartitionSpec
        from concourse.bass2jax import (
            _bass_exec_p, partition_id_tensor, install_neuronx_cc_hook)
        import warnings
        with warnings.catch_warnings():
            warnings.simplefilter("ignore")
            try:
                from jax.experimental.shard_map import shard_map
            except ImportError:
                from jax import shard_map

        install_neuronx_cc_hook()
        nc = _build_bass_fast()
        self.nc = nc
        partition_name = (nc.partition_id_tensor.name
                          if nc.partition_id_tensor else None)
        in_names, out_names, out_avals, zero_outs = [], [], [], []
        for alloc in nc.m.functions[0].allocations:
            if not isinstance(alloc, mybir.MemoryLocationSet):
                continue
            name = alloc.memorylocations[0].name
            if alloc.kind == "ExternalInput":
                if name != partition_name:
                    in_names.append(name)
            elif alloc.kind == "ExternalOutput":
                out_names.append(name)
                shape = tuple(alloc.tensor_shape)
                dtype = mybir.dt.np(alloc.dtype)
                out_avals.append(jax.core.ShapedArray(shape, dtype))
                zero_outs.append((shape, dtype))
        self.in_names = in_names
        self.out_names = out_names
        self.zero_outs = zero_outs
        n_params = len(in_names)
        n_outs = len(out_names)
        bind_names = tuple(in_names + out_names
                           + ([partition_name] if partition_name else []))

        def _body(*args):
            operands = list(args)
            if partition_name is not None:
                operands.append(partition_id_tensor())
            return tuple(_bass_exec_p.bind(
                *operands, out_avals=tuple(out_avals), in_names=bind_names,
                out_names=tuple(out_names),
                lowering_input_output_aliases=(),
                sim_require_finite=True, sim_require_nnan=True, nc=nc))

        devices = jax.devices()[:NCORES]
        assert len(devices) == NCORES
        mesh = Mesh(np.asarray(devices), ("core",))
        self.sharded = jax.jit(
            shard_map(_body, mesh=mesh,
                      in_specs=(PartitionSpec("core"),) * (n_params + n_outs),
                      out_specs=(PartitionSpec("core"),) * n_outs,
                      check_rep=False),
            keep_unused=True)
        # persistent device-resident zero output operands (not donated;
        # the program writes every output element, so reuse is safe)
        from jax.sharding import NamedSharding
        sh = NamedSharding(mesh, PartitionSpec("core"))
        self.dev_zeros = [
            jax.device_put(np.zeros((NCORES * s[0], *s[1:]), d), sh)
            for (s, d) in self.zero_outs]
        jax.block_until_ready(self.dev_zeros)

    def __call__(self, arrays_by_name):
        args = [arrays_by_name[n] for n in self.in_names]
        outs = self.sharded(*args, *self.dev_zeros)
        # asarray immediately after async dispatch: the d2h request
        # overlaps the dispatch round trip.
        return {n: np.asarray(o) for n, o in zip(self.out_names, outs)}


_COF_MEMO = {}
_SEL0_MEMO = {}
_DECODE_LUT = ((np.arange(256) - 0.5) * (1.0 / 254.0)).astype(np.float32)


def _cof_folded(ws):
    key = b"".join(np.ascontiguousarray(w).tobytes() for w in ws)
    hit = _COF_MEMO.get("cof")
    if hit is not None and hit[0] == key:
        return hit[1]
    coefs = [_coeffs(w) for w in ws]
    cof = np.stack([build_coef_sets(coefs, c) for c in range(NCORES)])
    # fold the u16 leaf dequantization into the L0 sets (a = q * s):
    s = np.float32(1.0 / 65535.0)
    cof[:, 0:4, :, 1] *= s          # c1 * s
    cof[:, 0:4, :, 2] *= s          # c2 * s
    cof[:, 0:4, :, 3] *= s * s      # c3 * s^2
    # fold the u8 output quantization (254*y + 0.5) into the L6 set:
    cof[:, 10] *= np.float32(254.0)
    cof[:, 10, :, 0] += np.float32(0.5)
    cof = np.ascontiguousarray(cof.reshape(NCORES * 11, 128, 4))
    _COF_MEMO["cof"] = (key, cof)
    return cof


def _kernel_fast(x, ws, widx):
    if "fast" not in _CACHE:
        _CACHE["fast"] = _FastRunner()
    runner = _CACHE["fast"]

    cof = _cof_folded(ws)
    skey = (id(widx[0]), id(widx[1]))
    sel0 = _SEL0_MEMO.get(skey)
    if sel0 is None:
        sel0 = build_sel0_all(widx).reshape(NCORES * 8, 75, 128)
        _SEL0_MEMO.clear()
        _SEL0_MEMO[skey] = sel0
    xq = (x * np.float32(65535.0) + np.float32(0.5)).astype(np.uint16)
    xg = np.ascontiguousarray(
        np.broadcast_to(xq, (NCORES,) + xq.shape)).reshape(NCORES * B, C, H, W)

    res = runner({"x": xg, "sel0": sel0, "coefs": cof})
    q = res["y"]                                          # [8*128, 784] u8
    y = _DECODE_LUT[q].reshape(NCORES, B, KLOC, P).transpose(1, 0, 2, 3)
    return np.ascontiguousarray(y.reshape(B, K, P, 1))


# ------------------------------------------------------------- fallback ----

def gather_leaves(x, left_idx, right_idx):
    """Host leaf gather with jax clamp semantics.

    Returns A, B: [NCORES, B, 4, 128, P] float32 where partition row of tile g
    is (k2=row//64 within pair {2g,2g+1}, s=row%64).
    """
    xf = np.ascontiguousarray(x).reshape(B, C * H * W)
    outs = []
    for idx in (left_idx, right_idx):
        h = np.clip(idx[..., 0], 0, H - 1).astype(np.int64)
        w = np.clip(idx[..., 1], 0, W - 1).astype(np.int64)
        c = np.clip(idx[..., 2], 0, C - 1).astype(np.int64)
        flat = c * (H * W) + h * W + w          # [K, P, S]
        flat = np.transpose(flat, (0, 2, 1))     # [K, S, P]
        g = xf[:, flat]                          # [B, K, S, P]
        g = g.reshape(B, NCORES, KLOC, S, P)
        g = np.transpose(g, (1, 0, 2, 3, 4))     # [cores, B, KLOC, S, P]
        outs.append(np.ascontiguousarray(
            g.reshape(NCORES, B, 4, 128, P).astype(np.float32)))
    return outs


def _build_bass_fallback():
    import concourse.mybir as mybir
    from concourse import bacc
    from concourse.tile import TileContext

    f32 = mybir.dt.float32
    Ident = mybir.ActivationFunctionType.Identity

    nc = bacc.Bacc("TRN2", target_bir_lowering=False, debug=False,
                   num_devices=NCORES)
    Ain_d = nc.dram_tensor("Ain", [B, 4, 128, P], f32,
                           kind="ExternalInput").ap()
    Bin_d = nc.dram_tensor("Bin", [B, 4, 128, P], f32,
                           kind="ExternalInput").ap()
    sel_d = nc.dram_tensor("sels", [24, 128, 128], f32, kind="ExternalInput").ap()
    cof_d = nc.dram_tensor("coefs", [11, 128, 4], f32, kind="ExternalInput").ap()
    y_d = nc.dram_tensor("y", [128, P], f32, kind="ExternalOutput").ap()

    with TileContext(nc) as tc:
        with (
            tc.tile_pool(name="const", bufs=1) as cpool,
            tc.tile_pool(name="ab", bufs=4) as ab,
            tc.tile_pool(name="uvw", bufs=4) as uvw,
            tc.tile_pool(name="s0", bufs=8) as s0p,
            tc.tile_pool(name="s1", bufs=4) as s1p,
            tc.tile_pool(name="s2", bufs=4) as s2p,
            tc.tile_pool(name="s3", bufs=4) as s3p,
            tc.tile_pool(name="s45", bufs=4) as s45p,
            tc.tile_pool(name="ps", bufs=2, space="PSUM") as ps,
        ):
            sel_t = []
            for m in range(24):
                t = cpool.tile([128, 128], f32, tag=f"sel{m}")
                nc.sync.dma_start(t[:], sel_d[m])
                sel_t.append(t)
            cof_t = []
            for m in range(11):
                t = cpool.tile([128, 4], f32, tag=f"cof{m}")
                nc.sync.dma_start(t[:], cof_d[m])
                cof_t.append(t)

            def sel(l, side, rel):
                return sel_t[(l - 1) * 4 + side * 2 + rel]

            def level_core(A_ap, B_ap, cs, out_tile):
                u = uvw.tile([128, P], f32, tag="u")
                v = uvw.tile([128, P], f32, tag="v")
                w = uvw.tile([128, P], f32, tag="w")
                nc.scalar.activation(u[:], A_ap, Ident,
                                     bias=cs[:, 2:3], scale=cs[:, 3:4])
                nc.scalar.activation(v[:], A_ap, Ident,
                                     bias=cs[:, 0:1], scale=cs[:, 1:2])
                nc.vector.tensor_mul(w[:], u[:], B_ap)
                nc.vector.tensor_add(out_tile[:], w[:], v[:])

            def level_mm(l, src0, src1, cs, out_tile):
                pA = ps.tile([128, P], f32, tag="pA")
                pB = ps.tile([128, P], f32, tag="pB")
                for (c0, c1) in COLS:
                    for rel, src in ((0, src0), (1, src1)):
                        nc.tensor.matmul(pA[:, c0:c1], sel(l, 0, rel)[:],
                                         src[:, c0:c1],
                                         start=(rel == 0), stop=(rel == 1))
                        nc.tensor.matmul(pB[:, c0:c1], sel(l, 1, rel)[:],
                                         src[:, c0:c1],
                                         start=(rel == 0), stop=(rel == 1))
                level_core(pA[:], pB[:], cs, out_tile)

            s2t = [None] * B
            s3t = [None] * 8
            s4t = [None] * 4
            s5t = [None] * 2
            for b in range(B):
                s0t = []
                for g in range(4):
                    At = ab.tile([128, P], f32, tag="Ain")
                    Bt = ab.tile([128, P], f32, tag="Bin")
                    nc.sync.dma_start(At[:], Ain_d[b, g])
                    nc.sync.dma_start(Bt[:], Bin_d[b, g])
                    st = s0p.tile([128, P], f32, tag="s0")
                    level_core(At[:], Bt[:], cof_t[g], st)
                    s0t.append(st)
                s1t = []
                for d in range(2):
                    st = s1p.tile([128, P], f32, tag="s1")
                    level_mm(1, s0t[2 * d], s0t[2 * d + 1], cof_t[4 + d], st)
                    s1t.append(st)
                st = s2p.tile([128, P], f32, tag="s2")
                level_mm(2, s1t[0], s1t[1], cof_t[6], st)
                s2t[b] = st
                if b % 2 == 1:
                    g3 = b // 2
                    st = s3p.tile([128, P], f32, tag="s3")
                    level_mm(3, s2t[b - 1], s2t[b], cof_t[7], st)
                    s3t[g3] = st
                if b % 4 == 3:
                    g4 = b // 4
                    st = s45p.tile([128, P], f32, tag="s4")
                    level_mm(4, s3t[2 * g4], s3t[2 * g4 + 1], cof_t[8], st)
                    s4t[g4] = st
                if b % 8 == 7:
                    g5 = b // 8
                    st = s45p.tile([128, P], f32, tag="s5")
                    level_mm(5, s4t[2 * g5], s4t[2 * g5 + 1], cof_t[9], st)
                    s5t[g5] = st
            yt = s45p.tile([128, P], f32, tag="s6")
            level_mm(6, s5t[0], s5t[1], cof_t[10], yt)
            nc.sync.dma_start(y_d[:], yt[:])
    nc.compile()
    return nc


def _kernel_fallback(x, ws, left_idx, right_idx):
    from concourse.bass_utils import run_bass_kernel_spmd

    if "fallback" not in _CACHE:
        _CACHE["fallback"] = _build_bass_fallback()
    nc = _CACHE["fallback"]

    coefs = [_coeffs(w) for w in ws]
    sels = build_sel_mats().reshape(24, 128, 128)
    csets = [build_coef_sets(coefs, c) for c in range(NCORES)]
    Ain, Bin = gather_leaves(x, left_idx, right_idx)
    in_maps = [
        {"Ain": Ain[c], "Bin": Bin[c], "sels": sels, "coefs": csets[c]}
        for c in range(NCORES)
    ]
    res = run_bass_kernel_spmd(nc, in_maps, core_ids=list(range(NCORES)))
    out = np.zeros((B, K, P, 1), dtype=np.float32)
    for c in range(NCORES):
        yc = res.results[c]["y"].reshape(B, KLOC, P)
        out[:, c * KLOC:(c + 1) * KLOC, :, 0] = yc
    return out


def kernel(x, w0, w1, w2, w3, w4, w5, w6, left_idx, right_idx):
    x = np.asarray(x, dtype=np.float32)
    ws = [np.asarray(w, dtype=np.float32) for w in (w0, w1, w2, w3, w4, w5, w6)]
    left_idx = np.asarray(left_idx)
    right_idx = np.asarray(right_idx)

    widx = detect_structure_memo(left_idx, right_idx)
    if widx is not None and x.min() >= 0.0 and x.max() <= 1.0:
        return _kernel_fast(x, ws, widx)
    return _kernel_fallback(x, ws, left_idx, right_idx)
